# revision 10
# baseline (speedup 1.0000x reference)
"""MoE feed-forward (256 experts, top-16, GLU) on 8 trn2 NeuronCores.

Strategy (expert parallel, per sharding hint):
  - Host: router (tiny matmul, softmax, top-k, renormalize) + builds the
    per-core dispatch: each core owns 32 experts; tokens routed to an
    expert are gathered (capacity C slots/expert, C derived from the
    actual routing) and pre-transposed so the device sees [D, C]
    activations per expert.
  - Device (per core, SPMD identical program, different data): for each
    of its 32 experts, stream ONE combined blob (x-gather + gate + up +
    down weights, f16, SBUF layout) from HBM — this is the memory-bound
    part — and run the grouped GEMMs (gate/up -> SiLU*up -> scale by
    routing weight -> down), writing [C, D] f16 per-expert output slots.
  - Host: combine = scatter-add the real slots back to token rows and
    sum partials over cores (cheap: ~5 MB/core).

The combined blob gives one dma_start per expert (128 x ~28.8KB
descriptors) so the sync queue issues only 33 input DMAs; output slots
are written f16 from the scalar engine's own queue so compute-dependent
writes never head-of-line block weight prefetch.
"""

import sys

import numpy as np

sys.path.insert(0, "/opt/trn_rl_repo")

import concourse.bacc as bacc
import concourse.mybir as mybir
import concourse.tile as tile
from concourse.bass_utils import run_bass_kernel_spmd
from concourse.masks import make_identity

# problem shapes (hardcoded per contract)
DIM = 1536
EXPERT_DIM = 384
NUM_EXPERTS = 256
TOP_K = 16
TOKENS = 512
N_CORES = 8
E_LOC = NUM_EXPERTS // N_CORES  # 32 experts per core
KC = DIM // 128  # 12 contraction chunks
HC = EXPERT_DIM // 128  # 3 chunks of the hidden dim

W_FREE = KC * EXPERT_DIM  # 4608 f16 elems per partition row per matrix

_COMPILED = {}  # CAP -> compiled program (program depends only on CAP)
_LAST_IN_MAPS = None  # stashed for test.py's separate timing run
_LAST_CAP = None


def _build_program(cap):
    f32 = mybir.dt.float32
    f16 = mybir.dt.float16
    nc = bacc.Bacc(
        "TRN2", target_bir_lowering=False, debug=False, num_devices=N_CORES
    )

    free = KC * cap + 3 * W_FREE
    x_off = 0
    g_off = KC * cap
    u_off = g_off + W_FREE
    d_off = u_off + W_FREE

    # per-core inputs, already in SBUF layout (partition-major, chunked)
    blob_d = nc.declare_dram_parameter("blob", [E_LOC, 128, free], f16, isOutput=False)
    cw_d = nc.declare_dram_parameter("cw", [cap, E_LOC], f32, isOutput=False)
    ys_d = nc.declare_dram_parameter("yslots", [E_LOC, cap, DIM], f16, isOutput=True)

    blob_a = blob_d.ap()
    cw_a = cw_d.ap()
    ys = ys_d.ap()

    with tile.TileContext(nc) as tc:
        with (
            tc.tile_pool(name="consts", bufs=1) as consts,
            tc.tile_pool(name="bpool", bufs=5) as bpool,
            tc.tile_pool(name="apool", bufs=2) as apool,
            tc.tile_pool(name="ypool", bufs=3) as ypool,
            tc.tile_pool(name="psgu", bufs=2, space="PSUM") as psgu,
            tc.tile_pool(name="pst", bufs=1, space="PSUM") as pstp,
            tc.tile_pool(name="psy", bufs=3, space="PSUM") as psyp,
        ):
            ident = consts.tile([128, 128], f16)
            make_identity(nc, ident)
            cw_sb = consts.tile([cap, E_LOC], f32)
            nc.sync.dma_start(out=cw_sb, in_=cw_a)

            # Software-pipelined with a 1-expert skew: the PE runs
            # gate/up(e) -> transpose(e-1) -> down(e-1), so the serial
            # sigmoid->mul->mul chain of expert e (scalar/vector) hides
            # under the PE's transpose+down work for expert e-1 instead
            # of stalling the in-order tensor queue.
            state = {}  # carried from iteration e to e+1

            def first_half(e):
                blob = bpool.tile([128, free], f16, tag="blob")
                nc.sync.dma_start(out=blob, in_=blob_a[e])

                psg = psgu.tile([cap, EXPERT_DIM], f32, tag="psg")
                psu = psgu.tile([cap, EXPERT_DIM], f32, tag="psu")
                for k in range(KC):
                    lhs = blob[:, x_off + k * cap : x_off + (k + 1) * cap]
                    nc.tensor.matmul(
                        psg[:], lhsT=lhs,
                        rhs=blob[:, g_off + k * EXPERT_DIM : g_off + (k + 1) * EXPERT_DIM],
                        start=(k == 0), stop=(k == KC - 1),
                    )
                    nc.tensor.matmul(
                        psu[:], lhsT=lhs,
                        rhs=blob[:, u_off + k * EXPERT_DIM : u_off + (k + 1) * EXPERT_DIM],
                        start=(k == 0), stop=(k == KC - 1),
                    )
                return blob, psg, psu

            def silu(e, psg, psu):
                # a = silu(g) * u : one ACT op + one DVE op
                sg = apool.tile([cap, EXPERT_DIM], f16, tag="sg")
                nc.scalar.activation(sg, psg, mybir.ActivationFunctionType.Silu)
                a_t = apool.tile([cap, EXPERT_DIM], f16, tag="a")
                nc.vector.tensor_mul(a_t, sg, psu)
                return a_t

            def second_half(e, blob, a_t):
                # aT: [C, 384] -> 3x [128, C]; all 3 transposes write
                # disjoint slices of one single-bank psum tile so they
                # don't serialize on buffer reuse
                ats = apool.tile([128, HC * cap], f16, tag="ats")
                pt = pstp.tile([128, HC, cap], f16, tag="pst")
                for h in range(HC):
                    nc.tensor.transpose(
                        pt[:, h, :], a_t[:, h * 128 : (h + 1) * 128], ident[:cap, :cap]
                    )
                    nc.vector.tensor_copy(ats[:, h * cap : (h + 1) * cap], pt[:, h, :])

                # down-proj into 3 single-bank psum chunks; h-outer order so
                # consecutive matmuls rotate across the 3 banks (full rate)
                pch = [
                    psyp.tile([cap, 512], f32, tag="psy", name=f"pch{s}")
                    for s in range(HC)
                ]
                for h in range(HC):
                    for s in range(HC):
                        nc.tensor.matmul(
                            pch[s][:], lhsT=ats[:, h * cap : (h + 1) * cap],
                            rhs=blob[:, d_off + h * DIM + s * 512 : d_off + h * DIM + (s + 1) * 512],
                            start=(h == 0), stop=(h == HC - 1),
                        )
                y_sb = ypool.tile([cap, DIM], f16, tag="ysb")
                for s in range(HC):
                    ych = y_sb[:, s * 512 : (s + 1) * 512]
                    nc.scalar.activation(
                        ych, pch[s],
                        mybir.ActivationFunctionType.Copy,
                        scale=cw_sb[:, e : e + 1],
                    )
                # write from the scalar queue: the data producer issues it,
                # so the sync queue (weight prefetch) never waits on compute
                nc.scalar.dma_start(out=ys[e], in_=y_sb)

            for e in range(E_LOC):
                blob, psg, psu = first_half(e)
                if state:
                    second_half(state["e"], state["blob"], state["a_t"])
                a_t = silu(e, psg, psu)
                state = {"e": e, "blob": blob, "a_t": a_t}
            second_half(state["e"], state["blob"], state["a_t"])

    nc.compile()
    return nc


def _route(x2d, Wr):
    """Host router: returns (sel [T,K] int, w [T,K] f32 renormalized)."""
    logits = x2d @ Wr.T
    m = logits.max(-1, keepdims=True)
    p = np.exp(logits - m)
    p /= p.sum(-1, keepdims=True)
    sel = np.argpartition(-p, TOP_K, axis=-1)[:, :TOP_K]
    w = np.take_along_axis(p, sel, axis=-1)
    w = w / w.sum(-1, keepdims=True)
    return sel, w.astype(np.float32)


def kernel(x, Wr, Wg, Wu, Wd, top_k):
    global _LAST_IN_MAPS, _LAST_CAP
    assert int(top_k) == TOP_K
    B, S, D = x.shape
    x2d = np.asarray(x, np.float32).reshape(-1, D)
    Wr = np.asarray(Wr, np.float32)

    sel, w = _route(x2d, Wr)

    # per-expert token lists; capacity = max count rounded up to 8
    toks = [[] for _ in range(NUM_EXPERTS)]
    wts = [[] for _ in range(NUM_EXPERTS)]
    for t in range(TOKENS):
        for j in range(TOP_K):
            e = int(sel[t, j])
            toks[e].append(t)
            wts[e].append(w[t, j])
    cap = max(8, -(-max(len(tl) for tl in toks) // 8) * 8)
    cap = min(cap, 128)
    for e in range(NUM_EXPERTS):  # truncate in the (unexpected) overflow case
        toks[e] = toks[e][:cap]
        wts[e] = wts[e][:cap]

    Wg16 = np.asarray(Wg).astype(np.float16)
    Wu16 = np.asarray(Wu).astype(np.float16)
    Wd16 = np.asarray(Wd).astype(np.float16)
    x16 = x2d.astype(np.float16)

    free = KC * cap + 3 * W_FREE
    in_maps = []
    idx_all = []
    for m in range(N_CORES):
        e0 = m * E_LOC
        idx = np.zeros((E_LOC, cap), np.int64)
        cnt = np.zeros(E_LOC, np.int64)
        cw = np.zeros((cap, E_LOC), np.float32)
        for le in range(E_LOC):
            tl = toks[e0 + le]
            n = len(tl)
            cnt[le] = n
            idx[le, :n] = tl
            cw[:n, le] = wts[e0 + le]
        idx_all.append((idx, cnt))

        blob = np.empty((E_LOC, 128, free), np.float16)
        # x gather -> [e, p, k, c]
        xg = x16[idx.reshape(-1)].reshape(E_LOC, cap, KC, 128)
        blob[:, :, : KC * cap] = (
            xg.transpose(0, 3, 2, 1).reshape(E_LOC, 128, KC * cap)
        )
        # weights -> SBUF layout: [e, p, k*h] with chunk-major free dim
        o = KC * cap
        blob[:, :, o : o + W_FREE] = (
            Wg16[e0 : e0 + E_LOC]
            .reshape(E_LOC, KC, 128, EXPERT_DIM)
            .transpose(0, 2, 1, 3)
            .reshape(E_LOC, 128, W_FREE)
        )
        o += W_FREE
        blob[:, :, o : o + W_FREE] = (
            Wu16[e0 : e0 + E_LOC]
            .reshape(E_LOC, KC, 128, EXPERT_DIM)
            .transpose(0, 2, 1, 3)
            .reshape(E_LOC, 128, W_FREE)
        )
        o += W_FREE
        blob[:, :, o : o + W_FREE] = (
            Wd16[e0 : e0 + E_LOC]
            .reshape(E_LOC, HC, 128, DIM)
            .transpose(0, 2, 1, 3)
            .reshape(E_LOC, 128, W_FREE)
        )

        in_maps.append({"blob": blob, "cw": cw})

    _LAST_IN_MAPS = in_maps
    _LAST_CAP = cap
    if cap not in _COMPILED:
        _COMPILED[cap] = _build_program(cap)
    nc = _COMPILED[cap]

    res = run_bass_kernel_spmd(nc, in_maps, core_ids=list(range(N_CORES)))

    y = np.zeros((TOKENS, DIM), np.float32)
    for m in range(N_CORES):
        ys = res.results[m]["yslots"].reshape(E_LOC, cap, DIM)
        idx, cnt = idx_all[m]
        for le in range(E_LOC):
            n = int(cnt[le])
            if n:
                np.add.at(y, idx[le, :n], ys[le, :n].astype(np.float32))
    return y.reshape(B, S, D).astype(np.float32)


# revision 17
# speedup vs baseline: 1.0143x; 1.0143x over previous
"""MoE feed-forward (256 experts, top-16, GLU) on 8 trn2 NeuronCores.

Strategy (expert parallel, per sharding hint):
  - Host: router (tiny matmul, softmax, top-k, renormalize) + builds the
    per-core dispatch: each core owns 32 experts; tokens routed to an
    expert are gathered (capacity C slots/expert, C derived from the
    actual routing) and pre-transposed so the device sees [D, C]
    activations per expert.
  - Device (per core, SPMD identical program, different data): for each
    of its 32 experts, stream ONE combined blob (x-gather + gate + up +
    down weights, f16, SBUF layout) from HBM — this is the memory-bound
    part — and run the grouped GEMMs (gate/up -> SiLU*up -> scale by
    routing weight -> down), writing [C, D] f16 per-expert output slots.
  - Host: combine = scatter-add the real slots back to token rows and
    sum partials over cores (cheap: ~5 MB/core).

The combined blob gives one dma_start per expert (128 x ~28.8KB
descriptors) so the sync queue issues only 33 input DMAs; output slots
are written f16 from the scalar engine's own queue so compute-dependent
writes never head-of-line block weight prefetch.
"""

import sys

import numpy as np

sys.path.insert(0, "/opt/trn_rl_repo")

import concourse.bacc as bacc
import concourse.mybir as mybir
import concourse.tile as tile
from concourse.bass_utils import run_bass_kernel_spmd
from concourse.masks import make_identity

# problem shapes (hardcoded per contract)
DIM = 1536
EXPERT_DIM = 384
NUM_EXPERTS = 256
TOP_K = 16
TOKENS = 512
N_CORES = 8
E_LOC = NUM_EXPERTS // N_CORES  # 32 experts per core
KC = DIM // 128  # 12 contraction chunks
HC = EXPERT_DIM // 128  # 3 chunks of the hidden dim

W_FREE = KC * EXPERT_DIM  # 4608 f16 elems per partition row per matrix

_COMPILED = {}  # CAP -> compiled program (program depends only on CAP)
_LAST_IN_MAPS = None  # stashed for test.py's separate timing run
_LAST_CAP = None


def _build_program(cap):
    f32 = mybir.dt.float32
    f16 = mybir.dt.float16
    nc = bacc.Bacc(
        "TRN2", target_bir_lowering=False, debug=False, num_devices=N_CORES
    )

    freeA = KC * cap + 2 * W_FREE  # x-gather + gate + up
    x_off = 0
    g_off = KC * cap
    u_off = g_off + W_FREE

    # per-core inputs, already in SBUF layout (partition-major, chunked);
    # two streams per expert so the A slot (x+gate+up) frees after the
    # gate/up matmuls and B (down) after the down matmuls
    ba_d = nc.declare_dram_parameter("blobA", [E_LOC, 128, freeA], f16, isOutput=False)
    bb_d = nc.declare_dram_parameter("blobB", [E_LOC, 128, W_FREE], f16, isOutput=False)
    cw_d = nc.declare_dram_parameter("cw", [cap, E_LOC], f32, isOutput=False)
    ys_d = nc.declare_dram_parameter("yslots", [E_LOC, cap, DIM], f16, isOutput=True)

    ba_a = ba_d.ap()
    bb_a = bb_d.ap()
    cw_a = cw_d.ap()
    ys = ys_d.ap()

    with tile.TileContext(nc) as tc:
        with (
            tc.tile_pool(name="consts", bufs=1) as consts,
            tc.tile_pool(name="bpoolA", bufs=5) as bpoolA,
            tc.tile_pool(name="bpoolB", bufs=6) as bpoolB,
            tc.tile_pool(name="apool", bufs=2) as apool,
            tc.tile_pool(name="ypool", bufs=3) as ypool,
            tc.tile_pool(name="psgu", bufs=2, space="PSUM") as psgu,
            tc.tile_pool(name="pst", bufs=1, space="PSUM") as pstp,
            tc.tile_pool(name="psy", bufs=3, space="PSUM") as psyp,
        ):
            ident = consts.tile([128, 128], f16)
            make_identity(nc, ident)
            cw_sb = consts.tile([cap, E_LOC], f32)
            nc.sync.dma_start(out=cw_sb, in_=cw_a)

            # Software-pipelined with a 1-expert skew: the PE runs
            # gate/up(e) -> transpose(e-1) -> down(e-1), so the serial
            # sigmoid->mul->mul chain of expert e (scalar/vector) hides
            # under the PE's transpose+down work for expert e-1 instead
            # of stalling the in-order tensor queue.
            state = {}  # carried from iteration e to e+1

            def first_half(e):
                ba = bpoolA.tile([128, freeA], f16, tag="ba")
                nc.sync.dma_start(out=ba, in_=ba_a[e])
                bb = bpoolB.tile([128, W_FREE], f16, tag="bb")
                nc.sync.dma_start(out=bb, in_=bb_a[e])

                psg = psgu.tile([cap, EXPERT_DIM], f32, tag="psg")
                psu = psgu.tile([cap, EXPERT_DIM], f32, tag="psu")
                for k in range(KC):
                    lhs = ba[:, x_off + k * cap : x_off + (k + 1) * cap]
                    nc.tensor.matmul(
                        psg[:], lhsT=lhs,
                        rhs=ba[:, g_off + k * EXPERT_DIM : g_off + (k + 1) * EXPERT_DIM],
                        start=(k == 0), stop=(k == KC - 1),
                    )
                    nc.tensor.matmul(
                        psu[:], lhsT=lhs,
                        rhs=ba[:, u_off + k * EXPERT_DIM : u_off + (k + 1) * EXPERT_DIM],
                        start=(k == 0), stop=(k == KC - 1),
                    )
                return bb, psg, psu

            def silu(e, psg, psu):
                # a = silu(g) * u : one ACT op + one DVE op
                sg = apool.tile([cap, EXPERT_DIM], f16, tag="sg")
                nc.scalar.activation(sg, psg, mybir.ActivationFunctionType.Silu)
                a_t = apool.tile([cap, EXPERT_DIM], f16, tag="a")
                nc.vector.tensor_mul(a_t, sg, psu)
                return a_t

            def second_half(e, bb, a_t):
                # aT: [C, 384] -> 3x [128, C]; all 3 transposes write
                # disjoint slices of one single-bank psum tile so they
                # don't serialize on buffer reuse
                ats = apool.tile([128, HC * cap], f16, tag="ats")
                pt = pstp.tile([128, HC, cap], f16, tag="pst")
                for h in range(HC):
                    nc.tensor.transpose(
                        pt[:, h, :], a_t[:, h * 128 : (h + 1) * 128], ident[:cap, :cap]
                    )
                    nc.vector.tensor_copy(ats[:, h * cap : (h + 1) * cap], pt[:, h, :])

                # down-proj into 3 single-bank psum chunks; h-outer order so
                # consecutive matmuls rotate across the 3 banks (full rate)
                pch = [
                    psyp.tile([cap, 512], f32, tag="psy", name=f"pch{s}")
                    for s in range(HC)
                ]
                for h in range(HC):
                    for s in range(HC):
                        nc.tensor.matmul(
                            pch[s][:], lhsT=ats[:, h * cap : (h + 1) * cap],
                            rhs=bb[:, h * DIM + s * 512 : h * DIM + (s + 1) * 512],
                            start=(h == 0), stop=(h == HC - 1),
                        )
                y_sb = ypool.tile([cap, DIM], f16, tag="ysb")
                for s in range(HC):
                    ych = y_sb[:, s * 512 : (s + 1) * 512]
                    nc.scalar.activation(
                        ych, pch[s],
                        mybir.ActivationFunctionType.Copy,
                        scale=cw_sb[:, e : e + 1],
                    )
                # write from the scalar queue: the data producer issues it,
                # so the sync queue (weight prefetch) never waits on compute
                nc.scalar.dma_start(out=ys[e], in_=y_sb)

            for e in range(E_LOC):
                bb, psg, psu = first_half(e)
                if state:
                    second_half(state["e"], state["bb"], state["a_t"])
                a_t = silu(e, psg, psu)
                state = {"e": e, "bb": bb, "a_t": a_t}
            second_half(state["e"], state["bb"], state["a_t"])

    nc.compile()
    return nc


def _route(x2d, Wr):
    """Host router: returns (sel [T,K] int, w [T,K] f32 renormalized)."""
    logits = x2d @ Wr.T
    m = logits.max(-1, keepdims=True)
    p = np.exp(logits - m)
    p /= p.sum(-1, keepdims=True)
    sel = np.argpartition(-p, TOP_K, axis=-1)[:, :TOP_K]
    w = np.take_along_axis(p, sel, axis=-1)
    w = w / w.sum(-1, keepdims=True)
    return sel, w.astype(np.float32)


def kernel(x, Wr, Wg, Wu, Wd, top_k):
    global _LAST_IN_MAPS, _LAST_CAP
    assert int(top_k) == TOP_K
    B, S, D = x.shape
    x2d = np.asarray(x, np.float32).reshape(-1, D)
    Wr = np.asarray(Wr, np.float32)

    sel, w = _route(x2d, Wr)

    # per-expert token lists; capacity = max count rounded up to 8
    toks = [[] for _ in range(NUM_EXPERTS)]
    wts = [[] for _ in range(NUM_EXPERTS)]
    for t in range(TOKENS):
        for j in range(TOP_K):
            e = int(sel[t, j])
            toks[e].append(t)
            wts[e].append(w[t, j])
    cap = max(8, -(-max(len(tl) for tl in toks) // 8) * 8)
    cap = min(cap, 128)
    for e in range(NUM_EXPERTS):  # truncate in the (unexpected) overflow case
        toks[e] = toks[e][:cap]
        wts[e] = wts[e][:cap]

    Wg16 = np.asarray(Wg).astype(np.float16)
    Wu16 = np.asarray(Wu).astype(np.float16)
    Wd16 = np.asarray(Wd).astype(np.float16)
    x16 = x2d.astype(np.float16)

    freeA = KC * cap + 2 * W_FREE
    in_maps = []
    idx_all = []
    for m in range(N_CORES):
        e0 = m * E_LOC
        idx = np.zeros((E_LOC, cap), np.int64)
        cnt = np.zeros(E_LOC, np.int64)
        cw = np.zeros((cap, E_LOC), np.float32)
        for le in range(E_LOC):
            tl = toks[e0 + le]
            n = len(tl)
            cnt[le] = n
            idx[le, :n] = tl
            cw[:n, le] = wts[e0 + le]
        idx_all.append((idx, cnt))

        ba = np.empty((E_LOC, 128, freeA), np.float16)
        # x gather -> [e, p, k, c]
        xg = x16[idx.reshape(-1)].reshape(E_LOC, cap, KC, 128)
        ba[:, :, : KC * cap] = (
            xg.transpose(0, 3, 2, 1).reshape(E_LOC, 128, KC * cap)
        )
        # weights -> SBUF layout: [e, p, k*h] with chunk-major free dim
        o = KC * cap
        ba[:, :, o : o + W_FREE] = (
            Wg16[e0 : e0 + E_LOC]
            .reshape(E_LOC, KC, 128, EXPERT_DIM)
            .transpose(0, 2, 1, 3)
            .reshape(E_LOC, 128, W_FREE)
        )
        o += W_FREE
        ba[:, :, o : o + W_FREE] = (
            Wu16[e0 : e0 + E_LOC]
            .reshape(E_LOC, KC, 128, EXPERT_DIM)
            .transpose(0, 2, 1, 3)
            .reshape(E_LOC, 128, W_FREE)
        )
        bb = np.ascontiguousarray(
            Wd16[e0 : e0 + E_LOC]
            .reshape(E_LOC, HC, 128, DIM)
            .transpose(0, 2, 1, 3)
            .reshape(E_LOC, 128, W_FREE)
        )

        in_maps.append({"blobA": ba, "blobB": bb, "cw": cw})

    _LAST_IN_MAPS = in_maps
    _LAST_CAP = cap
    if cap not in _COMPILED:
        _COMPILED[cap] = _build_program(cap)
    nc = _COMPILED[cap]

    res = run_bass_kernel_spmd(nc, in_maps, core_ids=list(range(N_CORES)))

    y = np.zeros((TOKENS, DIM), np.float32)
    for m in range(N_CORES):
        ys = res.results[m]["yslots"].reshape(E_LOC, cap, DIM)
        idx, cnt = idx_all[m]
        for le in range(E_LOC):
            n = int(cnt[le])
            if n:
                np.add.at(y, idx[le, :n], ys[le, :n].astype(np.float32))
    return y.reshape(B, S, D).astype(np.float32)


# revision 19
# speedup vs baseline: 1.0814x; 1.0661x over previous
"""MoE feed-forward (256 experts, top-16, GLU) on 8 trn2 NeuronCores.

Strategy (expert parallel, per sharding hint):
  - Host: router (tiny matmul, softmax, top-k, renormalize) + builds the
    per-core dispatch: each core owns 32 experts; tokens routed to an
    expert are gathered (capacity C slots/expert, C derived from the
    actual routing) and pre-transposed so the device sees [D, C]
    activations per expert.
  - Device (per core, SPMD identical program, different data): for each
    of its 32 experts, stream ONE combined blob (x-gather + gate + up +
    down weights, f16, SBUF layout) from HBM — this is the memory-bound
    part — and run the grouped GEMMs (gate/up -> SiLU*up -> scale by
    routing weight -> down), writing [C, D] f16 per-expert output slots.
  - Host: combine = scatter-add the real slots back to token rows and
    sum partials over cores (cheap: ~5 MB/core).

The combined blob gives one dma_start per expert (128 x ~28.8KB
descriptors) so the sync queue issues only 33 input DMAs; output slots
are written f16 from the scalar engine's own queue so compute-dependent
writes never head-of-line block weight prefetch.
"""

import sys

import numpy as np

sys.path.insert(0, "/opt/trn_rl_repo")

import concourse.bacc as bacc
import concourse.mybir as mybir
import concourse.tile as tile
from concourse.bass_utils import run_bass_kernel_spmd
from concourse.masks import make_identity

# problem shapes (hardcoded per contract)
DIM = 1536
EXPERT_DIM = 384
NUM_EXPERTS = 256
TOP_K = 16
TOKENS = 512
N_CORES = 8
E_LOC = NUM_EXPERTS // N_CORES  # 32 experts per core
KC = DIM // 128  # 12 contraction chunks
HC = EXPERT_DIM // 128  # 3 chunks of the hidden dim

W_FREE = KC * EXPERT_DIM  # 4608 f16 elems per partition row per matrix

_COMPILED = {}  # CAP -> compiled program (program depends only on CAP)
_LAST_IN_MAPS = None  # stashed for test.py's separate timing run
_LAST_CAP = None


def _build_program(cap):
    f32 = mybir.dt.float32
    f16 = mybir.dt.float16
    nc = bacc.Bacc(
        "TRN2", target_bir_lowering=False, debug=False, num_devices=N_CORES
    )

    freeA = KC * cap + 2 * W_FREE  # x-gather + gate + up
    x_off = 0
    g_off = KC * cap
    u_off = g_off + W_FREE

    # per-core inputs, already in SBUF layout (partition-major, chunked);
    # two streams per expert so the A slot (x+gate+up) frees after the
    # gate/up matmuls and B (down) after the down matmuls
    ba_d = nc.declare_dram_parameter("blobA", [E_LOC, 128, freeA], f16, isOutput=False)
    bb_d = nc.declare_dram_parameter("blobB", [E_LOC, 128, W_FREE], f16, isOutput=False)
    cw_d = nc.declare_dram_parameter("cw", [cap, E_LOC], f32, isOutput=False)
    ys_d = nc.declare_dram_parameter("yslots", [E_LOC, cap, DIM], f16, isOutput=True)

    ba_a = ba_d.ap()
    bb_a = bb_d.ap()
    cw_a = cw_d.ap()
    ys = ys_d.ap()

    with tile.TileContext(nc) as tc:
        with (
            tc.tile_pool(name="consts", bufs=1) as consts,
            tc.tile_pool(name="bpoolA", bufs=6) as bpoolA,
            tc.tile_pool(name="bpoolB", bufs=6) as bpoolB,
            tc.tile_pool(name="apool", bufs=2) as apool,
            tc.tile_pool(name="ypool", bufs=3) as ypool,
            tc.tile_pool(name="psgu", bufs=2, space="PSUM") as psgu,
            tc.tile_pool(name="pst", bufs=1, space="PSUM") as pstp,
            tc.tile_pool(name="psy", bufs=3, space="PSUM") as psyp,
        ):
            ident = consts.tile([128, 128], f16)
            make_identity(nc, ident)
            cw_sb = consts.tile([cap, E_LOC], f32)
            nc.sync.dma_start(out=cw_sb, in_=cw_a)

            # Software-pipelined with a 1-expert skew: the PE runs
            # gate/up(e) -> transpose(e-1) -> down(e-1), so the serial
            # sigmoid->mul->mul chain of expert e (scalar/vector) hides
            # under the PE's transpose+down work for expert e-1 instead
            # of stalling the in-order tensor queue.
            state = {}  # carried from iteration e to e+1

            def first_half(e):
                # A and B go down different hardware DMA queues (sync vs
                # gpsimd): each queue executes one DMA instruction's
                # descriptor set at a time, so two parallel streams keep
                # all 16 DMA engines fed across instruction boundaries
                ba = bpoolA.tile([128, freeA], f16, tag="ba")
                nc.sync.dma_start(out=ba, in_=ba_a[e])
                bb = bpoolB.tile([128, W_FREE], f16, tag="bb")
                nc.gpsimd.dma_start(out=bb, in_=bb_a[e])

                psg = psgu.tile([cap, EXPERT_DIM], f32, tag="psg")
                psu = psgu.tile([cap, EXPERT_DIM], f32, tag="psu")
                for k in range(KC):
                    lhs = ba[:, x_off + k * cap : x_off + (k + 1) * cap]
                    nc.tensor.matmul(
                        psg[:], lhsT=lhs,
                        rhs=ba[:, g_off + k * EXPERT_DIM : g_off + (k + 1) * EXPERT_DIM],
                        start=(k == 0), stop=(k == KC - 1),
                    )
                    nc.tensor.matmul(
                        psu[:], lhsT=lhs,
                        rhs=ba[:, u_off + k * EXPERT_DIM : u_off + (k + 1) * EXPERT_DIM],
                        start=(k == 0), stop=(k == KC - 1),
                    )
                return bb, psg, psu

            def silu(e, psg, psu):
                # a = silu(g) * u : one ACT op + one DVE op
                sg = apool.tile([cap, EXPERT_DIM], f16, tag="sg")
                nc.scalar.activation(sg, psg, mybir.ActivationFunctionType.Silu)
                a_t = apool.tile([cap, EXPERT_DIM], f16, tag="a")
                nc.vector.tensor_mul(a_t, sg, psu)
                return a_t

            def second_half(e, bb, a_t):
                # aT: [C, 384] -> 3x [128, C]; all 3 transposes write
                # disjoint slices of one single-bank psum tile so they
                # don't serialize on buffer reuse
                ats = apool.tile([128, HC * cap], f16, tag="ats")
                pt = pstp.tile([128, HC, cap], f16, tag="pst")
                for h in range(HC):
                    nc.tensor.transpose(
                        pt[:, h, :], a_t[:, h * 128 : (h + 1) * 128], ident[:cap, :cap]
                    )
                    nc.vector.tensor_copy(ats[:, h * cap : (h + 1) * cap], pt[:, h, :])

                # down-proj into 3 single-bank psum chunks; h-outer order so
                # consecutive matmuls rotate across the 3 banks (full rate)
                pch = [
                    psyp.tile([cap, 512], f32, tag="psy", name=f"pch{s}")
                    for s in range(HC)
                ]
                for h in range(HC):
                    for s in range(HC):
                        nc.tensor.matmul(
                            pch[s][:], lhsT=ats[:, h * cap : (h + 1) * cap],
                            rhs=bb[:, h * DIM + s * 512 : h * DIM + (s + 1) * 512],
                            start=(h == 0), stop=(h == HC - 1),
                        )
                y_sb = ypool.tile([cap, DIM], f16, tag="ysb")
                for s in range(HC):
                    ych = y_sb[:, s * 512 : (s + 1) * 512]
                    nc.scalar.activation(
                        ych, pch[s],
                        mybir.ActivationFunctionType.Copy,
                        scale=cw_sb[:, e : e + 1],
                    )
                # write from the scalar queue: the data producer issues it,
                # so the sync queue (weight prefetch) never waits on compute
                nc.scalar.dma_start(out=ys[e], in_=y_sb)

            for e in range(E_LOC):
                bb, psg, psu = first_half(e)
                if state:
                    second_half(state["e"], state["bb"], state["a_t"])
                a_t = silu(e, psg, psu)
                state = {"e": e, "bb": bb, "a_t": a_t}
            second_half(state["e"], state["bb"], state["a_t"])

    nc.compile()
    return nc


def _route(x2d, Wr):
    """Host router: returns (sel [T,K] int, w [T,K] f32 renormalized)."""
    logits = x2d @ Wr.T
    m = logits.max(-1, keepdims=True)
    p = np.exp(logits - m)
    p /= p.sum(-1, keepdims=True)
    sel = np.argpartition(-p, TOP_K, axis=-1)[:, :TOP_K]
    w = np.take_along_axis(p, sel, axis=-1)
    w = w / w.sum(-1, keepdims=True)
    return sel, w.astype(np.float32)


def kernel(x, Wr, Wg, Wu, Wd, top_k):
    global _LAST_IN_MAPS, _LAST_CAP
    assert int(top_k) == TOP_K
    B, S, D = x.shape
    x2d = np.asarray(x, np.float32).reshape(-1, D)
    Wr = np.asarray(Wr, np.float32)

    sel, w = _route(x2d, Wr)

    # per-expert token lists; capacity = max count rounded up to 8
    toks = [[] for _ in range(NUM_EXPERTS)]
    wts = [[] for _ in range(NUM_EXPERTS)]
    for t in range(TOKENS):
        for j in range(TOP_K):
            e = int(sel[t, j])
            toks[e].append(t)
            wts[e].append(w[t, j])
    cap = max(8, -(-max(len(tl) for tl in toks) // 8) * 8)
    cap = min(cap, 128)
    for e in range(NUM_EXPERTS):  # truncate in the (unexpected) overflow case
        toks[e] = toks[e][:cap]
        wts[e] = wts[e][:cap]

    Wg16 = np.asarray(Wg).astype(np.float16)
    Wu16 = np.asarray(Wu).astype(np.float16)
    Wd16 = np.asarray(Wd).astype(np.float16)
    x16 = x2d.astype(np.float16)

    freeA = KC * cap + 2 * W_FREE
    in_maps = []
    idx_all = []
    for m in range(N_CORES):
        e0 = m * E_LOC
        idx = np.zeros((E_LOC, cap), np.int64)
        cnt = np.zeros(E_LOC, np.int64)
        cw = np.zeros((cap, E_LOC), np.float32)
        for le in range(E_LOC):
            tl = toks[e0 + le]
            n = len(tl)
            cnt[le] = n
            idx[le, :n] = tl
            cw[:n, le] = wts[e0 + le]
        idx_all.append((idx, cnt))

        ba = np.empty((E_LOC, 128, freeA), np.float16)
        # x gather -> [e, p, k, c]
        xg = x16[idx.reshape(-1)].reshape(E_LOC, cap, KC, 128)
        ba[:, :, : KC * cap] = (
            xg.transpose(0, 3, 2, 1).reshape(E_LOC, 128, KC * cap)
        )
        # weights -> SBUF layout: [e, p, k*h] with chunk-major free dim
        o = KC * cap
        ba[:, :, o : o + W_FREE] = (
            Wg16[e0 : e0 + E_LOC]
            .reshape(E_LOC, KC, 128, EXPERT_DIM)
            .transpose(0, 2, 1, 3)
            .reshape(E_LOC, 128, W_FREE)
        )
        o += W_FREE
        ba[:, :, o : o + W_FREE] = (
            Wu16[e0 : e0 + E_LOC]
            .reshape(E_LOC, KC, 128, EXPERT_DIM)
            .transpose(0, 2, 1, 3)
            .reshape(E_LOC, 128, W_FREE)
        )
        bb = np.ascontiguousarray(
            Wd16[e0 : e0 + E_LOC]
            .reshape(E_LOC, HC, 128, DIM)
            .transpose(0, 2, 1, 3)
            .reshape(E_LOC, 128, W_FREE)
        )

        in_maps.append({"blobA": ba, "blobB": bb, "cw": cw})

    _LAST_IN_MAPS = in_maps
    _LAST_CAP = cap
    if cap not in _COMPILED:
        _COMPILED[cap] = _build_program(cap)
    nc = _COMPILED[cap]

    res = run_bass_kernel_spmd(nc, in_maps, core_ids=list(range(N_CORES)))

    y = np.zeros((TOKENS, DIM), np.float32)
    for m in range(N_CORES):
        ys = res.results[m]["yslots"].reshape(E_LOC, cap, DIM)
        idx, cnt = idx_all[m]
        for le in range(E_LOC):
            n = int(cnt[le])
            if n:
                np.add.at(y, idx[le, :n], ys[le, :n].astype(np.float32))
    return y.reshape(B, S, D).astype(np.float32)
